# revision 13
# baseline (speedup 1.0000x reference)
"""CrossAttention kernel for 8 Trainium2 NeuronCores.

Sharding: batch (2) x head-groups (4 groups of 4 heads) = 8 shards.
Core c handles batch b = c//4 and heads [4*(c%4), 4*(c%4)+4).

Per-core dataflow (everything "transposed": feature dims on partitions,
sequence on the free axis, so no on-device transposes are needed):
  qT[e,n]  = Wq_s.T @ x_b.T     (lhsT = Wq_s tiles, rhs = xT tiles)
  kT[e,k]  = Wk_s.T @ ctx_b.T
  v[k,e]   = ctx_b @ Wv_s       (lhsT = ctxT tiles, rhs = Wv_s)   + ones col
  S^T[k,n] = k_h.T x q_h        per head (K=64 contraction)
  P^T      = exp(SCALE * S^T)   on ScalarE, PSUM -> SBUF bf16
  out_aug[65,n] = [v_h | 1].T @ P^T  accumulated over k tiles;
                  row 64 = softmax denominators
  out_h    = out_aug[0:64] * (1/denom) + bv_h
  finalT[e,n] = Wp_s.T @ concat_h(out_h)  (+ bproj/4)
Host sums the 4 head-group partials per batch and transposes back.

Hardware note: non-PE engine instructions can encode only ONE sync wait,
so the program keeps a single PSUM pool for the whole kernel (pool
transitions spray WAR waits onto first accessors) and "observer" ops make
each engine see the consts DMA before any compute joins.
"""

import sys

sys.path.insert(0, "/opt/trn_rl_repo")

import numpy as np
import ml_dtypes

import concourse.bass as bass
from concourse import mybir
from concourse.tile import TileContext
from concourse.bass_utils import run_bass_kernel_spmd

BF16 = ml_dtypes.bfloat16
F32 = np.float32

D = 1024
N = 2048
K = 2048
H_PER_CORE = 4
HD = 64
E = H_PER_CORE * HD  # 256 features per core
SCALE = HD ** -0.5

_CACHED = {}


def _build_program():
    nc = bass.Bass()
    dt = mybir.dt

    xT_h = nc.dram_tensor("xT", [D, N], dt.bfloat16, kind="ExternalInput")
    cT_h = nc.dram_tensor("ctxT", [D, K], dt.bfloat16, kind="ExternalInput")
    wq_h = nc.dram_tensor("wq", [D, E], dt.bfloat16, kind="ExternalInput")
    wk_h = nc.dram_tensor("wk", [D, E], dt.bfloat16, kind="ExternalInput")
    wv_h = nc.dram_tensor("wv", [D, E], dt.bfloat16, kind="ExternalInput")
    wp_h = nc.dram_tensor("wp", [E, D], dt.bfloat16, kind="ExternalInput")
    consts_h = nc.dram_tensor("consts", [128, 16], dt.float32, kind="ExternalInput")
    outT_h = nc.dram_tensor("outT", [D, N], dt.float32, kind="ExternalOutput")

    DT = D // 128  # 8 d tiles
    KT = K // 128  # 16 k tiles
    NC4 = N // 512  # 4 n chunks of 512

    with TileContext(nc) as tc:
        with tc.tile_pool(name="persist", bufs=1) as persist, \
             tc.tile_pool(name="obs", bufs=1) as obs, \
             tc.tile_pool(name="psb", bufs=3) as psb_pool, \
             tc.tile_pool(name="norm", bufs=2) as norm_pool, \
             tc.tile_pool(name="fin", bufs=3) as fin_pool, \
             tc.tile_pool(name="dscr", bufs=2, space="DRAM") as dscr_pool, \
             tc.tile_pool(name="ps", bufs=2, space="PSUM") as psp, \
             tc.tile_pool(name="po", bufs=1, space="PSUM") as pso:

            # ---- persistent SBUF tensors ----
            xT = persist.tile([128, DT, N], dt.bfloat16, tag="xT")
            cT = persist.tile([128, DT, K], dt.bfloat16, tag="cT")
            wq = persist.tile([128, DT, E], dt.bfloat16, tag="wq")
            wk = persist.tile([128, DT, E], dt.bfloat16, tag="wk")
            wv = persist.tile([128, DT, E], dt.bfloat16, tag="wv")
            wp = persist.tile([64, H_PER_CORE, D], dt.bfloat16, tag="wp")
            cst = persist.tile([128, 16], dt.float32, tag="cst")
            qT = persist.tile([128, 2, N], dt.bfloat16, tag="qT")
            kT = persist.tile([128, 2, K], dt.bfloat16, tag="kT")
            vsb = persist.tile([128, KT, H_PER_CORE * 65], dt.bfloat16, tag="vsb")
            osb = persist.tile([64, H_PER_CORE, N], dt.bfloat16, tag="osb")

            nc.sync.dma_start(out=cst, in_=consts_h[:, :])
            nc.sync.dma_start(out=xT, in_=xT_h.rearrange("(t p) n -> p t n", p=128))
            nc.sync.dma_start(out=cT, in_=cT_h.rearrange("(t p) n -> p t n", p=128))
            nc.sync.dma_start(out=wq, in_=wq_h.rearrange("(t p) e -> p t e", p=128))
            nc.sync.dma_start(out=wk, in_=wk_h.rearrange("(t p) e -> p t e", p=128))
            nc.sync.dma_start(out=wv, in_=wv_h.rearrange("(t p) e -> p t e", p=128))
            nc.sync.dma_start(out=wp, in_=wp_h.rearrange("(h p) e -> p h e", p=64))

            # each compute engine observes the consts DMA up front so later
            # instructions never need a DMA wait alongside a compute wait
            ob_a = obs.tile([128, 1], dt.float32, tag="ob_a")
            ob_v = obs.tile([128, 1], dt.float32, tag="ob_v")
            ob_g = obs.tile([128, 1], dt.float32, tag="ob_g")
            nc.scalar.copy(out=ob_a, in_=cst[:, 15:16])
            nc.vector.tensor_copy(out=ob_v, in_=cst[:, 15:16])
            nc.gpsimd.tensor_copy(out=ob_g, in_=cst[:, 15:16])
            # PE observes each input DMA via a throwaway ldweights, so real
            # matmuls never need a DMA wait next to a compute wait
            for t in (xT, cT, wq, wk, wv, wp):
                nc.tensor.ldweights(t[:, 0, 0:1])

            # ---- Phase A: projections ----
            # qT / kT: [e-tile 128, n-chunk 512] accumulated over 8 d tiles
            for w_sb, src, dst, bcol in ((wq, xT, qT, 0), (wk, cT, kT, 2)):
                for et in range(2):
                    for nch in range(NC4):
                        ps = psp.tile([128, 512], dt.float32, tag="s")
                        for dtile in range(DT):
                            nc.tensor.matmul(
                                ps,
                                w_sb[:, dtile, et * 128:(et + 1) * 128],
                                src[:, dtile, nch * 512:(nch + 1) * 512],
                                start=(dtile == 0),
                                stop=(dtile == DT - 1),
                            )
                        nc.scalar.add(
                            out=dst[:, et, nch * 512:(nch + 1) * 512],
                            in_=ps,
                            add=cst[:, bcol + et:bcol + et + 1],
                        )
            # v in natural [k, e] layout, augmented with a ones column/head
            for kt in range(KT):
                ps = psp.tile([128, E], dt.float32, tag="s")
                for dtile in range(DT):
                    nc.tensor.matmul(
                        ps,
                        cT[:, dtile, kt * 128:(kt + 1) * 128],
                        wv[:, dtile, :],
                        start=(dtile == 0),
                        stop=(dtile == DT - 1),
                    )
                vdst = vsb[:, kt, :].rearrange("p (h e) -> p h e", h=H_PER_CORE)
                nc.vector.tensor_copy(
                    out=vdst[:, :, 0:64],
                    in_=ps.rearrange("p (h e) -> p h e", h=H_PER_CORE),
                )
                nc.vector.memset(vdst[:, :, 64:65], 1.0)

            # ---- Phase B: attention per head ----
            for h in range(H_PER_CORE):
                et, off = h // 2, (h % 2) * 64
                oaug = pso.tile([65, N], dt.float32, tag="o")
                for kt in range(KT):
                    psb = psb_pool.tile([128, N], dt.bfloat16, tag="psb")
                    for half in range(2):
                        s_ps = psp.tile([128, 1024], dt.float32, tag="s")
                        for j in range(2):
                            n0 = half * 1024 + j * 512
                            nc.tensor.matmul(
                                s_ps[:, j * 512:(j + 1) * 512],
                                kT[off:off + 64, et, kt * 128:(kt + 1) * 128],
                                qT[off:off + 64, et, n0:n0 + 512],
                                start=True,
                                stop=True,
                            )
                        nc.scalar.activation(
                            out=psb[:, half * 1024:(half + 1) * 1024],
                            in_=s_ps,
                            func=mybir.ActivationFunctionType.Exp,
                            bias=cst[:, 15:16],
                            scale=SCALE,
                        )
                    for j in range(NC4):
                        nc.tensor.matmul(
                            oaug[:, j * 512:(j + 1) * 512],
                            vsb[:, kt, h * 65:(h + 1) * 65],
                            psb[:, j * 512:(j + 1) * 512],
                            start=(kt == 0),
                            stop=(kt == KT - 1),
                        )
                # normalize: out_h = oaug[0:64] / denom + bv_h
                rrow = norm_pool.tile([65, N], dt.float32, tag="rrow")
                nc.vector.reciprocal(out=rrow[64:65, :], in_=oaug[64:65, :])
                # broadcast the recip row to 64 partitions via a DRAM bounce
                # (zero-stride partition APs are only legal on DRAM sources)
                rscr = dscr_pool.tile([1, N], dt.float32, tag="rscr")
                nc.sync.dma_start(out=rscr, in_=rrow[64:65, :])
                bcast = norm_pool.tile([64, N], dt.float32, tag="bcast")
                nc.gpsimd.dma_start(
                    out=bcast,
                    in_=bass.AP(tensor=rscr.tensor, offset=rscr.offset,
                                ap=[[0, 64]] + list(rscr.ap[1:])),
                )
                nc.vector.tensor_mul(
                    out=osb[:, h, :], in0=oaug[0:64, :], in1=bcast
                )
                nc.vector.tensor_scalar_add(
                    out=osb[:, h, :],
                    in0=osb[:, h, :],
                    scalar1=cst[0:64, 4 + h:5 + h],
                )

            # ---- Phase C: output projection ----
            for et in range(DT):
                for nch in range(NC4):
                    ps = psp.tile([128, 512], dt.float32, tag="s")
                    for h in range(H_PER_CORE):
                        nc.tensor.matmul(
                            ps,
                            wp[:, h, et * 128:(et + 1) * 128],
                            osb[:, h, nch * 512:(nch + 1) * 512],
                            start=(h == 0),
                            stop=(h == H_PER_CORE - 1),
                        )
                    fin = fin_pool.tile([128, 512], dt.float32, tag="fin")
                    # toucher: absorbs the WAR-vs-output-DMA wait on its own,
                    # so the real bias-add carries only the PE wait
                    nc.scalar.copy(out=fin[0:1, 0:1], in_=cst[0:1, 15:16])
                    nc.scalar.add(out=fin, in_=ps, add=cst[:, 8 + et:9 + et])
                    nc.sync.dma_start(
                        out=outT_h[et * 128:(et + 1) * 128,
                                   nch * 512:(nch + 1) * 512],
                        in_=fin,
                    )

    _strip_self_waits(nc)
    _legalize_sync(nc)
    return nc


_LEGAL_N = [0]


def _mk_evsem(engine, waits, updates):
    _LEGAL_N[0] += 1
    return mybir.InstEventSemaphore(
        name=f"legal-ev-{_LEGAL_N[0]}",
        engine=engine,
        ins=[],
        outs=[],
        sync_info=mybir.SyncInfo(on_wait=waits, on_update=updates),
    )


def _legalize_sync(nc):
    """This walrus build encodes at most ONE sync wait per engine
    instruction and cannot codegen the SEM RANGE_CLEAR InstISA in Tile's
    tail. Split extra waits into standalone single-wait EventSemaphore
    instructions (the raw-bass idiom) and replace the range clear with
    per-semaphore EVSEM zero-writes."""
    for fn in nc.m.functions:
        for blk in fn.blocks:
            out = []
            for inst in blk.instructions:
                si = inst.sync_info
                if type(inst).__name__ == "InstISA" and str(inst.engine) == "EngineType.Pool":
                    # Tile's tail sem RANGE_CLEAR — NRT resets sem state per
                    # execution anyway (verified empirically), so drop it.
                    continue
                if si and si.on_wait and len(si.on_wait) > 1:
                    waits = list(si.on_wait)
                    for w in waits[:-1]:
                        out.append(_mk_evsem(inst.engine, [w], []))
                    si.on_wait = [waits[-1]]
                    inst.sync_info = si
                out.append(inst)
            blk.instructions = out


# Non-PE compute-engine instructions can encode only one sync wait in their
# ISA struct. ACT/DVE/GpSimd execute strictly in order, so a wait on the
# engine's own completion semaphore (Tile emits these for same-engine
# WAW/WAR slot reuse) is always already satisfied — drop them.
_STRIPPABLE = {
    "EngineType.Activation": "Activation_",
    "EngineType.DVE": "DVE_",
    "EngineType.Pool": "Pool_",
    "EngineType.PE": "PE_",
}
_STRIP_TYPES = (
    "InstActivation", "InstTensorTensor", "InstTensorScalarPtr",
    "InstTensorCopy", "InstReciprocal", "InstMemset",
    "InstPartitionBroadcast", "InstTensorReduce", "InstMatmult",
)


def _strip_self_waits(nc):
    for fn in nc.m.functions:
        for blk in fn.blocks:
            for inst in blk.instructions:
                if type(inst).__name__ not in _STRIP_TYPES:
                    continue
                pre = _STRIPPABLE.get(str(inst.engine))
                si = inst.sync_info
                if not pre or not si or not si.on_wait or len(si.on_wait) < 2:
                    continue
                keep = [w for w in si.on_wait if not w.ant_name.startswith(pre)]
                if len(keep) != len(si.on_wait):
                    si.on_wait = keep
                    inst.sync_info = si


def _make_in_maps(x, ctx, Wq, bq, Wkv, bkv, Wproj, bproj):
    in_maps = []
    for c in range(8):
        b, hg = c // 4, c % 4
        s = slice(hg * E, (hg + 1) * E)
        consts = np.zeros((128, 16), F32)
        consts[:, 0] = bq[s][0:128]
        consts[:, 1] = bq[s][128:256]
        consts[:, 2] = bkv[s][0:128]
        consts[:, 3] = bkv[s][128:256]
        for h in range(4):
            consts[0:64, 4 + h] = bkv[D + hg * E + h * 64: D + hg * E + (h + 1) * 64]
        for et in range(8):
            consts[:, 8 + et] = bproj[et * 128:(et + 1) * 128] / 4.0
        in_maps.append({
            "xT": np.ascontiguousarray(x[b].T).astype(BF16),
            "ctxT": np.ascontiguousarray(ctx[b].T).astype(BF16),
            "wq": Wq[:, s].astype(BF16),
            "wk": Wkv[:, s].astype(BF16),
            "wv": Wkv[:, D + hg * E: D + (hg + 1) * E].astype(BF16),
            "wp": Wproj[s, :].astype(BF16),
            "consts": consts,
        })
    return in_maps


def run(inputs, trace=False, **kw):
    if "nc" not in _CACHED:
        _CACHED["nc"] = _build_program()
    nc = _CACHED["nc"]
    in_maps = _make_in_maps(**inputs)
    res = run_bass_kernel_spmd(nc, in_maps, list(range(8)), trace=trace, **kw)
    out = np.zeros((2, N, D), F32)
    for c in range(8):
        out[c // 4] += np.asarray(res.results[c]["outT"], F32).T
    return out, res


def kernel(**inputs):
    out, _ = run(inputs)
    return out


# revision 18
# speedup vs baseline: 1.2046x; 1.2046x over previous
"""CrossAttention kernel for 8 Trainium2 NeuronCores.

Sharding: batch (2) x head-groups (4 groups of 4 heads) = 8 shards.
Core c handles batch b = c//4 and heads [4*(c%4), 4*(c%4)+4).

Per-core dataflow (everything "transposed": feature dims on partitions,
sequence on the free axis, so no on-device transposes are needed):
  qT[e,n]  = Wq_s.T @ x_b.T     (lhsT = Wq_s tiles, rhs = xT tiles)
  kT[e,k]  = Wk_s.T @ ctx_b.T
  v[k,e]   = ctx_b @ Wv_s       (lhsT = ctxT tiles, rhs = Wv_s)   + ones col
  S^T[k,n] = k_h.T x q_h        per head (K=64 contraction)
  P^T      = exp(SCALE * S^T)   on ScalarE, PSUM -> SBUF bf16
  out_aug[65,n] = [v_h | 1].T @ P^T  accumulated over k tiles;
                  row 64 = softmax denominators
  out_h    = out_aug[0:64] * (1/denom) + bv_h
  finalT[e,n] = Wp_s.T @ concat_h(out_h)  (+ bproj/4)
Host sums the 4 head-group partials per batch and transposes back.

Hardware note: non-PE engine instructions can encode only ONE sync wait,
so the program keeps a single PSUM pool for the whole kernel (pool
transitions spray WAR waits onto first accessors) and "observer" ops make
each engine see the consts DMA before any compute joins.
"""

import sys

sys.path.insert(0, "/opt/trn_rl_repo")

import numpy as np
import ml_dtypes

import concourse.bass as bass
from concourse import mybir
from concourse.tile import TileContext
from concourse.bass_utils import run_bass_kernel_spmd

BF16 = ml_dtypes.bfloat16
F32 = np.float32

D = 1024
N = 2048
K = 2048
H_PER_CORE = 4
HD = 64
E = H_PER_CORE * HD  # 256 features per core
SCALE = HD ** -0.5

_CACHED = {}


def _build_program():
    nc = bass.Bass()
    dt = mybir.dt

    xT_h = nc.dram_tensor("xT", [D, N], dt.bfloat16, kind="ExternalInput")
    cT_h = nc.dram_tensor("ctxT", [D, K], dt.bfloat16, kind="ExternalInput")
    wq_h = nc.dram_tensor("wq", [D, E], dt.bfloat16, kind="ExternalInput")
    wk_h = nc.dram_tensor("wk", [D, E], dt.bfloat16, kind="ExternalInput")
    wv_h = nc.dram_tensor("wv", [D, E], dt.bfloat16, kind="ExternalInput")
    wp_h = nc.dram_tensor("wp", [E, D], dt.bfloat16, kind="ExternalInput")
    consts_h = nc.dram_tensor("consts", [128, 16], dt.float32, kind="ExternalInput")
    outT_h = nc.dram_tensor("outT", [D, N], dt.float32, kind="ExternalOutput")

    DT = D // 128  # 8 d tiles
    KT = K // 128  # 16 k tiles
    NC4 = N // 512  # 4 n chunks of 512

    with TileContext(nc) as tc:
        with tc.tile_pool(name="persist", bufs=1) as persist, \
             tc.tile_pool(name="obs", bufs=1) as obs, \
             tc.tile_pool(name="psb", bufs=3) as psb_pool, \
             tc.tile_pool(name="norm", bufs=2) as norm_pool, \
             tc.tile_pool(name="fin", bufs=3) as fin_pool, \
             tc.tile_pool(name="dscr", bufs=2, space="DRAM") as dscr_pool, \
             tc.tile_pool(name="ps", bufs=2, space="PSUM") as psp, \
             tc.tile_pool(name="po", bufs=1, space="PSUM") as pso:

            # ---- persistent SBUF tensors ----
            xT = persist.tile([128, DT, N], dt.bfloat16, tag="xT")
            cT = persist.tile([128, DT, K], dt.bfloat16, tag="cT")
            wq = persist.tile([128, DT, E], dt.bfloat16, tag="wq")
            wk = persist.tile([128, DT, E], dt.bfloat16, tag="wk")
            wv = persist.tile([128, DT, E], dt.bfloat16, tag="wv")
            wp = persist.tile([64, H_PER_CORE, D], dt.bfloat16, tag="wp")
            cst = persist.tile([128, 16], dt.float32, tag="cst")
            qT = persist.tile([128, 2, N], dt.bfloat16, tag="qT")
            kT = persist.tile([128, 2, K], dt.bfloat16, tag="kT")
            vsb = persist.tile([128, KT, H_PER_CORE * 65], dt.bfloat16, tag="vsb")
            osb = persist.tile([64, H_PER_CORE, N], dt.bfloat16, tag="osb")

            nc.sync.dma_start(out=cst, in_=consts_h[:, :])
            nc.sync.dma_start(out=xT, in_=xT_h.rearrange("(t p) n -> p t n", p=128))
            nc.sync.dma_start(out=cT, in_=cT_h.rearrange("(t p) n -> p t n", p=128))
            nc.sync.dma_start(out=wq, in_=wq_h.rearrange("(t p) e -> p t e", p=128))
            nc.sync.dma_start(out=wk, in_=wk_h.rearrange("(t p) e -> p t e", p=128))
            nc.sync.dma_start(out=wv, in_=wv_h.rearrange("(t p) e -> p t e", p=128))
            nc.sync.dma_start(out=wp, in_=wp_h.rearrange("(h p) e -> p h e", p=64))



            # ---- Phase A: projections ----
            # qT / kT: [e-tile 128, n-chunk 512] accumulated over 8 d tiles
            for w_sb, src, dst, bcol in ((wq, xT, qT, 0), (wk, cT, kT, 2)):
                for et in range(2):
                    for nch in range(NC4):
                        ps = psp.tile([128, 512], dt.float32, tag="s")
                        for dtile in range(DT):
                            nc.tensor.matmul(
                                ps,
                                w_sb[:, dtile, et * 128:(et + 1) * 128],
                                src[:, dtile, nch * 512:(nch + 1) * 512],
                                start=(dtile == 0),
                                stop=(dtile == DT - 1),
                            )
                        nc.vector.tensor_scalar_add(
                            out=dst[:, et, nch * 512:(nch + 1) * 512],
                            in0=ps,
                            scalar1=cst[:, bcol + et:bcol + et + 1],
                        )
            # v in natural [k, e] layout, augmented with a ones column/head
            for kt in range(KT):
                ps = psp.tile([128, E], dt.float32, tag="s")
                for dtile in range(DT):
                    nc.tensor.matmul(
                        ps,
                        cT[:, dtile, kt * 128:(kt + 1) * 128],
                        wv[:, dtile, :],
                        start=(dtile == 0),
                        stop=(dtile == DT - 1),
                    )
                vdst = vsb[:, kt, :].rearrange("p (h e) -> p h e", h=H_PER_CORE)
                nc.vector.tensor_copy(
                    out=vdst[:, :, 0:64],
                    in_=ps.rearrange("p (h e) -> p h e", h=H_PER_CORE),
                )
                nc.vector.memset(vdst[:, :, 64:65], 1.0)

            # ---- Phase B: attention per head ----
            for h in range(H_PER_CORE):
                et, off = h // 2, (h % 2) * 64
                oaug = pso.tile([65, N], dt.float32, tag="o")
                for kt in range(KT):
                    psb = psb_pool.tile([128, N], dt.bfloat16, tag="psb")
                    for half in range(2):
                        s_ps = psp.tile([128, 1024], dt.float32, tag="s")
                        for j in range(2):
                            n0 = half * 1024 + j * 512
                            nc.tensor.matmul(
                                s_ps[:, j * 512:(j + 1) * 512],
                                kT[off:off + 64, et, kt * 128:(kt + 1) * 128],
                                qT[off:off + 64, et, n0:n0 + 512],
                                start=True,
                                stop=True,
                            )
                        nc.scalar.activation(
                            out=psb[:, half * 1024:(half + 1) * 1024],
                            in_=s_ps,
                            func=mybir.ActivationFunctionType.Exp,
                            bias=cst[:, 15:16],
                            scale=SCALE,
                        )
                    for j in range(NC4):
                        nc.tensor.matmul(
                            oaug[:, j * 512:(j + 1) * 512],
                            vsb[:, kt, h * 65:(h + 1) * 65],
                            psb[:, j * 512:(j + 1) * 512],
                            start=(kt == 0),
                            stop=(kt == KT - 1),
                        )
                # copy PSUM out early so the next head's attn@v can reuse
                # the banks without waiting on the normalize chain
                oasb = norm_pool.tile([65, N], dt.float32, tag="oasb")
                nc.vector.tensor_copy(out=oasb, in_=oaug)
                # normalize: out_h = oaug[0:64] / denom + bv_h.  The recip
                # row is bounced through DRAM so it can be (a) reshaped to
                # [128, 16] to use all DVE lanes and (b) partition-broadcast
                # back (zero-stride partition APs are only legal on DRAM).
                rscr = dscr_pool.tile([1, N], dt.float32, tag="rscr")
                nc.sync.dma_start(out=rscr, in_=oasb[64:65, :])
                rden = norm_pool.tile([128, N // 128], dt.float32, tag="rden")
                nc.sync.dma_start(
                    out=rden, in_=rscr[0, :].rearrange("(p j) -> p j", p=128)
                )
                nc.vector.reciprocal(out=rden, in_=rden)
                rscr2 = dscr_pool.tile([1, N], dt.float32, tag="rscr2")
                nc.sync.dma_start(
                    out=rscr2[0, :].rearrange("(p j) -> p j", p=128), in_=rden
                )
                bcast = norm_pool.tile([64, N], dt.float32, tag="bcast")
                nc.gpsimd.dma_start(
                    out=bcast,
                    in_=bass.AP(tensor=rscr2.tensor, offset=rscr2.offset,
                                ap=[[0, 64]] + list(rscr2.ap[1:])),
                )
                nc.vector.tensor_mul(
                    out=osb[:, h, :], in0=oasb[0:64, :], in1=bcast
                )
                nc.vector.tensor_scalar_add(
                    out=osb[:, h, :],
                    in0=osb[:, h, :],
                    scalar1=cst[0:64, 4 + h:5 + h],
                )

            # ---- Phase C: output projection ----
            for et in range(DT):
                for nch in range(NC4):
                    ps = psp.tile([128, 512], dt.float32, tag="s")
                    for h in range(H_PER_CORE):
                        nc.tensor.matmul(
                            ps,
                            wp[:, h, et * 128:(et + 1) * 128],
                            osb[:, h, nch * 512:(nch + 1) * 512],
                            start=(h == 0),
                            stop=(h == H_PER_CORE - 1),
                        )
                    fin = fin_pool.tile([128, 512], dt.float32, tag="fin")
                    nc.vector.tensor_scalar_add(
                        out=fin, in0=ps, scalar1=cst[:, 8 + et:9 + et]
                    )
                    nc.sync.dma_start(
                        out=outT_h[et * 128:(et + 1) * 128,
                                   nch * 512:(nch + 1) * 512],
                        in_=fin,
                    )

    _strip_self_waits(nc)
    _legalize_sync(nc)
    return nc


_LEGAL_N = [0]


def _mk_evsem(engine, waits, updates):
    _LEGAL_N[0] += 1
    return mybir.InstEventSemaphore(
        name=f"legal-ev-{_LEGAL_N[0]}",
        engine=engine,
        ins=[],
        outs=[],
        sync_info=mybir.SyncInfo(on_wait=waits, on_update=updates),
    )


def _legalize_sync(nc):
    """This walrus build encodes at most ONE sync wait per engine
    instruction and cannot codegen the SEM RANGE_CLEAR InstISA in Tile's
    tail. Split extra waits into standalone single-wait EventSemaphore
    instructions (the raw-bass idiom) and replace the range clear with
    per-semaphore EVSEM zero-writes."""
    for fn in nc.m.functions:
        for blk in fn.blocks:
            out = []
            for inst in blk.instructions:
                si = inst.sync_info
                if type(inst).__name__ == "InstISA" and str(inst.engine) == "EngineType.Pool":
                    # Tile's tail sem RANGE_CLEAR — NRT resets sem state per
                    # execution anyway (verified empirically), so drop it.
                    continue
                if si and si.on_wait and len(si.on_wait) > 1:
                    waits = list(si.on_wait)
                    for w in waits[:-1]:
                        out.append(_mk_evsem(inst.engine, [w], []))
                    si.on_wait = [waits[-1]]
                    inst.sync_info = si
                out.append(inst)
            blk.instructions = out


# Non-PE compute-engine instructions can encode only one sync wait in their
# ISA struct. ACT/DVE/GpSimd execute strictly in order, so a wait on the
# engine's own completion semaphore (Tile emits these for same-engine
# WAW/WAR slot reuse) is always already satisfied — drop them.
_STRIPPABLE = {
    "EngineType.Activation": "Activation_",
    "EngineType.DVE": "DVE_",
    "EngineType.Pool": "Pool_",
    "EngineType.PE": "PE_",
}
_STRIP_TYPES = (
    "InstActivation", "InstTensorTensor", "InstTensorScalarPtr",
    "InstTensorCopy", "InstReciprocal", "InstMemset",
    "InstPartitionBroadcast", "InstTensorReduce", "InstMatmult",
)


def _strip_self_waits(nc):
    for fn in nc.m.functions:
        for blk in fn.blocks:
            for inst in blk.instructions:
                if type(inst).__name__ not in _STRIP_TYPES:
                    continue
                pre = _STRIPPABLE.get(str(inst.engine))
                si = inst.sync_info
                if not pre or not si or not si.on_wait or len(si.on_wait) < 2:
                    continue
                keep = [w for w in si.on_wait if not w.ant_name.startswith(pre)]
                if len(keep) != len(si.on_wait):
                    si.on_wait = keep
                    inst.sync_info = si


def _make_in_maps(x, ctx, Wq, bq, Wkv, bkv, Wproj, bproj):
    in_maps = []
    for c in range(8):
        b, hg = c // 4, c % 4
        s = slice(hg * E, (hg + 1) * E)
        consts = np.zeros((128, 16), F32)
        consts[:, 0] = bq[s][0:128]
        consts[:, 1] = bq[s][128:256]
        consts[:, 2] = bkv[s][0:128]
        consts[:, 3] = bkv[s][128:256]
        for h in range(4):
            consts[0:64, 4 + h] = bkv[D + hg * E + h * 64: D + hg * E + (h + 1) * 64]
        for et in range(8):
            consts[:, 8 + et] = bproj[et * 128:(et + 1) * 128] / 4.0
        in_maps.append({
            "xT": np.ascontiguousarray(x[b].T).astype(BF16),
            "ctxT": np.ascontiguousarray(ctx[b].T).astype(BF16),
            "wq": Wq[:, s].astype(BF16),
            "wk": Wkv[:, s].astype(BF16),
            "wv": Wkv[:, D + hg * E: D + (hg + 1) * E].astype(BF16),
            "wp": Wproj[s, :].astype(BF16),
            "consts": consts,
        })
    return in_maps


def run(inputs, trace=False, **kw):
    if "nc" not in _CACHED:
        _CACHED["nc"] = _build_program()
    nc = _CACHED["nc"]
    in_maps = _make_in_maps(**inputs)
    res = run_bass_kernel_spmd(nc, in_maps, list(range(8)), trace=trace, **kw)
    out = np.zeros((2, N, D), F32)
    for c in range(8):
        out[c // 4] += np.asarray(res.results[c]["outT"], F32).T
    return out, res


def kernel(**inputs):
    out, _ = run(inputs)
    return out
